# revision 2
# baseline (speedup 1.0000x reference)
"""GCN classifier with metrics — TRN2 Bass kernel (8 NeuronCores, SPMD), v2.

Strategy (see baseline docstring for problem statement):
  - Nodes partitioned contiguously across 8 cores (12500/core, 98 tiles of
    128). Tiles grouped into 4 chunks (25/25/24/24); table windows are
    "chunk q of every core" so chunked AllGathers unlock gather windows
    progressively.
  - Feature-major dataflow: per tile, h/r live as [64 feats, 128 nodes];
    aggregation matmuls use the gathered messages as the stationary operand
    (lhsT [128 slots, 64]) against on-device-generated one-hot S [128 slots,
    128 dst] (moving), accumulating aggT [64, 128] in PSUM, then added into
    a full-shard SBUF accumulator.
  - S is generated by DVE is_equal against a packed iota table (no S DMA).
  - Self-loop contributions are added locally (agg accumulator initialized
    with the tile's own scaled features) — no gather slots for self loops.
  - Slot layout per (window): per-(tile,window) allocation = max count over
    cores (SPMD-uniform structure), packed contiguously (~6% padding).
  - global_mean_pool: per-tile indicator matmul accumulates poolT [16, 256]
    in PSUM; AllReduce [16, 256]; head computed redundantly per core with
    host-folded metrics contribution and 1/count scaling.
"""
import sys
import numpy as np

sys.path.insert(0, "/opt/trn_rl_repo")

import ml_dtypes
import concourse.bass as bass
import concourse.bacc as bacc
import concourse.mybir as mybir
import concourse.tile as tile
from concourse.bass_utils import run_bass_kernel_spmd
from concourse.library_config import mlp as mlp_lib

BF16 = ml_dtypes.bfloat16
F32 = mybir.dt.float32
BF = mybir.dt.bfloat16
I16 = mybir.dt.int16

N = 100_000
E = 1_600_000
G = 256
CIN = 128
NCLS = 10
NCORES = 8
SHARD = 12_500
SHARD_PAD = 12_544
NT = 98
H1 = 64
H2 = 16
NWIN = 4
MAX_CALL_BLK = 8            # 1024 idx per dma_gather (hard ucode limit)
CH_TILES = np.array([25, 25, 24, 24])
CH_START = np.array([0, 25, 50, 74])
CH_ROWS = CH_TILES * 128                   # rows per core-chunk
WIN_ROWS = CH_ROWS * 8
WIN_BASE = np.zeros(4, np.int64)
WIN_BASE[1:] = np.cumsum(WIN_ROWS)[:-1]
TROWS = int(WIN_ROWS.sum())                # 100352
CHUNK_OF_TILE = np.repeat(np.arange(4), CH_TILES)
XB = 8                                     # x tiles per load


def _wrap_idx(idx):
    """[n] int16 (n % 128 == 0) -> [128, n//16] wrapped + replicated layout."""
    n = len(idx)
    w = idx.reshape(n // 16, 16).T.astype(np.int16)
    return np.tile(w, (8, 1))


def _build_structure(src, dst):
    """Slot schedule (SPMD-uniform) + per-core idx/dstpart data."""
    nodes = np.arange(N, dtype=np.int64)
    ncore = nodes // SHARD
    nloc = nodes % SHARD
    nt = nloc // 128
    npart = nloc % 128
    nq = CHUNK_OF_TILE[nt]
    nrel = ncore * CH_ROWS[nq] + (nt - CH_START[nq]) * 128 + npart

    ecore = dst // SHARD
    dloc = dst - ecore * SHARD
    td = dloc // 128
    pd = dloc % 128
    ws = nq[src]
    rels = nrel[src]

    bucket = td * NWIN + ws                           # [E] in [0, 392)
    gb = ecore * (NT * NWIN) + bucket
    cnt = np.bincount(gb, minlength=NCORES * NT * NWIN).reshape(
        NCORES, NT * NWIN)
    alloc = cnt.max(axis=0)                           # [392] t-major, w-minor
    allocw = alloc.reshape(NT, NWIN)

    off_tw = np.zeros((NT, NWIN), np.int64)
    off_tw[1:] = np.cumsum(allocw, axis=0)[:-1]
    slotw = allocw.sum(axis=0)
    nblk = -(-slotw // 128)
    slotw_pad = nblk * 128
    wslot_base = np.zeros(NWIN, np.int64)
    wslot_base[1:] = np.cumsum(slotw_pad)[:-1]
    totslot = int(slotw_pad.sum())

    # segments (w, b, t, start, stop), ordered by (w, b, t)
    segs = []
    for w in range(NWIN):
        for t in range(NT):
            a = int(allocw[t, w])
            if a == 0:
                continue
            o = int(off_tw[t, w])
            b0, b1 = o // 128, (o + a - 1) // 128
            for b in range(b0, b1 + 1):
                segs.append((w, b, t, b == b0, b == b1))
    segs.sort(key=lambda s: (s[0], s[1], s[2]))
    nseg_tot = len(segs)

    # calls
    calls = []
    seg_ptr = 0
    for w in range(NWIN):
        for b0 in range(0, int(nblk[w]), MAX_CALL_BLK):
            nb = min(MAX_CALL_BLK, int(nblk[w]) - b0)
            lo = seg_ptr
            while (seg_ptr < nseg_tot and segs[seg_ptr][0] == w
                   and segs[seg_ptr][1] < b0 + nb):
                seg_ptr += 1
            calls.append(dict(w=w, b0=b0, nb=nb, seg_lo=lo, seg_hi=seg_ptr))
    assert seg_ptr == nseg_tot
    nsegc_max = max(c["seg_hi"] - c["seg_lo"] for c in calls)

    # uniform tile-of-slot map
    tos = np.full(totslot, -1, np.int64)
    for w in range(NWIN):
        fill = np.repeat(np.arange(NT), allocw[:, w])
        tos[wslot_base[w]:wslot_base[w] + slotw[w]] = fill
    seg_w = np.array([s[0] for s in segs])
    seg_b = np.array([s[1] for s in segs])
    seg_t = np.array([s[2] for s in segs])
    seg_slot0 = wslot_base[seg_w] + seg_b * 128
    seg_rows = seg_slot0[:, None] + np.arange(128)[None, :]   # [nseg, 128]
    tos_seg = tos[seg_rows]

    bstart = np.zeros((NCORES, NT * NWIN), np.int64)
    bstart[:, 1:] = np.cumsum(cnt, axis=1)[:, :-1]

    per_core = []
    for c in range(NCORES):
        sel = ecore == c
        b_e = bucket[sel]
        key = b_e * 32768 + rels[sel]
        order = np.argsort(key, kind="stable")
        b_s = b_e[order]
        r_s = rels[sel][order]
        p_s = pd[sel][order]
        w_s = ws[sel][order]
        t_s = td[sel][order]
        rank = np.arange(len(b_s)) - bstart[c][b_s]
        pos = wslot_base[w_s] + off_tw[t_s, w_s] + rank
        idx_arr = np.zeros(totslot, np.int64)
        dst_arr = np.full(totslot, -1.0, np.float32)
        idx_arr[pos] = r_s
        dst_arr[pos] = p_s
        dp = dst_arr[seg_rows]                         # [nseg, 128]
        dp = np.where(tos_seg == seg_t[:, None], dp, -1.0)
        per_core.append({
            "idx": _wrap_idx(idx_arr.astype(np.int16)),
            "dstp": np.ascontiguousarray(
                np.repeat(dp.T, 2, axis=1)).astype(BF16),
        })

    sched = dict(calls=calls, segs=segs, nblk=nblk, wslot_base=wslot_base,
                 slotw_pad=slotw_pad, nseg_tot=nseg_tot, nsegc_max=nsegc_max,
                 totslot=totslot, allocw=allocw, off_tw=off_tw)
    return sched, per_core


def _build_program(sched, alpha1, alpha2):
    nc = bacc.Bacc("TRN2", target_bir_lowering=False, debug=False,
                   num_devices=NCORES, num_swdge_queues=4)
    nseg_tot = sched["nseg_tot"]
    nsegc_max = sched["nsegc_max"]
    totslot = sched["totslot"]
    slotw_pad = sched["slotw_pad"]
    wslot_base = sched["wslot_base"]

    def inp(name, shape, dt=F32):
        return nc.declare_dram_parameter(name, shape, dt, isOutput=False)

    xTb = inp("xTb", [CIN, SHARD_PAD], BF)
    dinv_rep_d = inp("dinv_rep", [H1, SHARD_PAD], BF)
    idxT = inp("idx", [128, totslot // 16], I16)
    dstp_d = inp("dstp", [128, 2 * nseg_tot], BF)
    iota128_d = inp("iota128", [128, 128], BF)
    iota256_d = inp("iota256", [128, 256], BF)
    batchc_d = inp("batchc", [128, NT])
    identb_d = inp("identb", [128, 128], BF)
    identf_d = inp("identf", [128, 128])
    W1_d = inp("W1", [CIN, H1], BF)
    Wr1_d = inp("Wr1", [CIN, H1], BF)
    W2_d = inp("W2", [H1, H2], BF)
    Wr2_d = inp("Wr2", [H1, H2], BF)
    b1c_d = inp("b1c", [H1, 1])
    br1c_d = inp("br1c", [H1, 1])
    b2c_d = inp("b2c", [H2, 1])
    br2c_d = inp("br2c", [H2, 1])
    Wf1a_d = inp("Wf1a", [16, 80])
    Wf2_d = inp("Wf2", [80, NCLS])
    mc_d = inp("mc_col", [80, 1])
    bf2c_d = inp("bf2c", [NCLS, 1])
    invc_d = inp("invc_rep", [16, 256])
    out = nc.declare_dram_parameter("out", [G, NCLS], F32, isOutput=True)

    SILU = mybir.ActivationFunctionType.Silu
    COPY = mybir.ActivationFunctionType.Copy
    ISEQ = mybir.AluOpType.is_equal
    MULT = mybir.AluOpType.mult

    with tile.TileContext(nc) as tc:
        with tc.tile_pool(name="const", bufs=1) as constp, \
             tc.tile_pool(name="store", bufs=1) as storep, \
             tc.tile_pool(name="xw", bufs=2) as xwp, \
             tc.tile_pool(name="idxw", bufs=2) as idxwp, \
             tc.tile_pool(name="mt", bufs=13) as mtp, \
             tc.tile_pool(name="sg", bufs=6) as sgp, \
             tc.tile_pool(name="ep", bufs=3) as epp, \
             tc.tile_pool(name="hd", bufs=1) as hdp, \
             tc.tile_pool(name="cb", bufs=1) as cbp, \
             tc.tile_pool(name="dram", bufs=1, space="DRAM") as dram, \
             tc.tile_pool(name="ps_agg", bufs=3, space="PSUM") as ps_agg, \
             tc.tile_pool(name="ps_mm", bufs=2, space="PSUM") as ps_mm, \
             tc.tile_pool(name="ps_t", bufs=2, space="PSUM") as ps_t, \
             tc.tile_pool(name="ps_pool", bufs=1, space="PSUM") as ps_pool:

            nc.gpsimd.load_library(mlp_lib)

            def ld(ap_src, shape, dt=F32, tag=None):
                t = constp.tile(shape, dt, tag=tag or ap_src.tensor.name)
                nc.sync.dma_start(out=t[:], in_=ap_src)
                return t

            dinv_rep = ld(dinv_rep_d[:], [H1, SHARD_PAD], BF)
            dstp_sb = ld(dstp_d[:], [128, 2 * nseg_tot], BF)
            iota128 = ld(iota128_d[:], [128, 128], BF)
            iota256 = ld(iota256_d[:], [128, 256], BF)
            batchc = ld(batchc_d[:], [128, NT])
            identb = ld(identb_d[:], [128, 128], BF)
            identf = ld(identf_d[:], [128, 128])
            W1_sb = ld(W1_d[:], [CIN, H1], BF)
            Wr1_sb = ld(Wr1_d[:], [CIN, H1], BF)
            W2_sb = ld(W2_d[:], [H1, H2], BF)
            Wr2_sb = ld(Wr2_d[:], [H1, H2], BF)
            b1c = ld(b1c_d[:], [H1, 1])
            br1c = ld(br1c_d[:], [H1, 1])
            b2c = ld(b2c_d[:], [H2, 1])
            br2c = ld(br2c_d[:], [H2, 1])
            Wf1a = ld(Wf1a_d[:], [16, 80])
            Wf2_sb = ld(Wf2_d[:], [80, NCLS])
            mc_col = ld(mc_d[:], [80, 1])
            bf2c = ld(bf2c_d[:], [NCLS, 1])
            invc = ld(invc_d[:], [16, 256])

            agg_sb = storep.tile([H1, NT * 128], BF, tag="agg")
            r1T = storep.tile([H1, NT * 128], BF, tag="r1T")
            r2T = storep.tile([H2, NT * 128], BF, tag="r2T")

            h1s_chunk = [dram.tile([int(CH_ROWS[q]), 128], BF,
                                   tag=f"h1s{q}", name=f"h1s{q}")
                         for q in range(4)]
            h2s_chunk = [dram.tile([int(CH_ROWS[q]), 128], BF,
                                   tag=f"h2s{q}", name=f"h2s{q}")
                         for q in range(4)]
            table1 = [dram.tile([int(WIN_ROWS[q]), 128], BF,
                                tag=f"table1_{q}", name=f"table1_{q}",
                                addr_space="Shared") for q in range(4)]
            table2 = [dram.tile([int(WIN_ROWS[q]), 128], BF,
                                tag=f"table2_{q}", name=f"table2_{q}",
                                addr_space="Shared") for q in range(4)]
            pool_in = dram.tile([H2, 256], F32, tag="pool_in")
            pool_out = dram.tile([H2, 256], F32, tag="pool_out")

            # ---------------- stage 0 ----------------
            for q in range(4):
                t0q, ntq = int(CH_START[q]), int(CH_TILES[q])
                chunkbuf = cbp.tile([128, 29 * 128], BF, tag="cb1")
                for t0 in range(t0q, t0q + ntq, XB):
                    ntl = min(XB, t0q + ntq - t0)
                    xw = xwp.tile([128, XB * 128], BF, tag="xw")
                    nc.scalar.dma_start(
                        out=xw[:, :ntl * 128],
                        in_=xTb[:, t0 * 128:(t0 + ntl) * 128])
                    for a in range(ntl):
                        t = t0 + a
                        tr = slice(t * 128, (t + 1) * 128)
                        dv = dinv_rep[:, tr]
                        xcol = xw[:, a * 128:(a + 1) * 128]
                        ps_h = ps_mm.tile([128, 256], F32, tag="mm",
                                          name=f"psh{t}")
                        nc.tensor.matmul(out=ps_h[0:H1, 0:128], lhsT=W1_sb[:],
                                         rhs=xcol, start=True, stop=True)
                        ps_r = ps_mm.tile([128, 256], F32, tag="mm",
                                          name=f"psr{t}")
                        nc.tensor.matmul(out=ps_r[0:H1, 0:128], lhsT=Wr1_sb[:],
                                         rhs=xcol, start=True, stop=True)
                        nc.vector.tensor_tensor(out=agg_sb[:, tr],
                                                in0=ps_h[0:H1, 0:128],
                                                in1=dv, op=MULT)
                        h1tb = epp.tile([H1, 128], BF, tag="h1tb")
                        nc.scalar.activation(out=h1tb[:], in_=agg_sb[:, tr],
                                             func=COPY)
                        nc.scalar.activation(out=r1T[:, tr],
                                             in_=ps_r[0:H1, 0:128],
                                             func=SILU, bias=br1c[:])
                        nc.scalar.activation(out=r1T[:, tr],
                                             in_=r1T[:, tr], func=COPY,
                                             scale=float(alpha1))
                        tp = ps_t.tile([128, 256], BF, tag="tpb",
                                       name=f"tpa{t}")
                        nc.tensor.transpose(out=tp[:, 0:H1], in_=h1tb[:],
                                            identity=identb[0:H1, 0:H1])
                        tloc = t - t0q
                        nc.scalar.activation(
                            out=chunkbuf[:, tloc * 128:tloc * 128 + H1],
                            in_=tp[:, 0:H1], func=COPY)
                nc.sync.dma_start(
                    out=h1s_chunk[q][:].rearrange("(a p) c -> p a c", p=128),
                    in_=chunkbuf[:, :ntq * 128])
                nc.gpsimd.collective_compute(
                    "AllGather", mybir.AluOpType.bypass,
                    replica_groups=[list(range(NCORES))],
                    ins=[h1s_chunk[q][:]],
                    outs=[table1[q][:]])

            qctr = [0]

            # per-seg quad first/last flags
            NQ = (NT + 3) // 4
            seg_qfirst = [False] * len(sched["segs"])
            seg_qlast = [False] * len(sched["segs"])
            seen = {}
            for si, (sw, sb, st_, _a, _b) in enumerate(sched["segs"]):
                key = (sw, st_ // 4)
                if key not in seen:
                    seg_qfirst[si] = True
                seen[key] = si
            for key, si in seen.items():
                seg_qlast[si] = True

            def run_layer(table, epilogue, win_pre=None):
                live = {}          # quad -> psum tile
                for w in range(NWIN):
                    if win_pre is not None and w >= 1:
                        win_pre(w)
                    wcalls = [c for c in sched["calls"] if c["w"] == w]
                    idxw_sb = idxwp.tile(
                        [128, int(max(slotw_pad)) // 16], I16, tag="idxw")
                    c0 = int(wslot_base[w]) // 16
                    nc.sync.dma_start(
                        out=idxw_sb[:, :int(slotw_pad[w]) // 16],
                        in_=idxT[:, c0:c0 + int(slotw_pad[w]) // 16])
                    for call in wcalls:
                        b0, nb = call["b0"], call["nb"]
                        nsegc = call["seg_hi"] - call["seg_lo"]
                        mt = mtp.tile([128, MAX_CALL_BLK * 128], BF, tag="mt")
                        nc.gpsimd.dma_gather(
                            mt[:, :nb * 128].rearrange(
                                "p (b d) -> p b d", d=128),
                            table[w][:],
                            idxw_sb[:, b0 * 8:b0 * 8 + nb * 8],
                            nb * 128, nb * 128, 128,
                            queue_num=qctr[0] % 4,
                        )
                        qctr[0] += 1
                        sg = sgp.tile([128, 128 * nsegc_max], BF, tag="sg")
                        sview = sg[:, :128 * nsegc].rearrange(
                            "p (s d) -> p s d", d=128)
                        sview2 = sg[:, :128 * nsegc].rearrange(
                            "p (s d2 j) -> p s d2 j", d2=64, j=2)
                        nc.vector.tensor_tensor(
                            out=sview2,
                            in0=dstp_sb[:, 2 * call["seg_lo"]:
                                        2 * call["seg_hi"]]
                                .rearrange("p (s j) -> p s j", j=2)
                                .unsqueeze(2)
                                .to_broadcast([128, nsegc, 64, 2]),
                            in1=iota128[:]
                                .rearrange("p (d2 j) -> p d2 j", j=2)
                                .unsqueeze(1)
                                .to_broadcast([128, nsegc, 64, 2]),
                            op=ISEQ)
                        for sloc in range(nsegc):
                            si = call["seg_lo"] + sloc
                            (sw, sb, st_, _a, _b) = sched["segs"][si]
                            g, m = st_ // 4, st_ % 4
                            qw = min(4, NT - g * 4) * 128
                            gr = slice(g * 512, g * 512 + qw)
                            if seg_qfirst[si]:
                                live[g] = ps_agg.tile([H1, 512], F32,
                                                      tag="pagg",
                                                      name=f"agg{sw}_{g}")
                                nc.tensor.matmul(
                                    out=live[g][:, 0:qw],
                                    lhsT=identb[0:H1, 0:H1],
                                    rhs=agg_sb[:, gr],
                                    start=True, stop=False,
                                    skip_group_check=True)
                            nc.tensor.matmul(
                                out=live[g][:, m * 128:(m + 1) * 128],
                                lhsT=mt[:, (sb - b0) * 128:
                                        (sb - b0) * 128 + H1],
                                rhs=sview[:, sloc, :],
                                start=False, stop=seg_qlast[si],
                                skip_group_check=True)
                            if seg_qlast[si]:
                                nc.scalar.activation(out=agg_sb[:, gr],
                                                     in_=live[g][:, 0:qw],
                                                     func=COPY)
                                del live[g]
                                if w == NWIN - 1:
                                    epilogue(g)
                assert not live

            # ---------------- layer 1 ----------------
            epi1_state = {"done": np.zeros(4, np.int64),
                          "cb": [None] * 4, "tp": None, "tpn": 0, "tp0": 0}

            def flush_tp(q):
                st = epi1_state
                if st["tp"] is None:
                    return
                k0, k = st["tp0"], st["tpn"]
                nc.scalar.activation(
                    out=st["cb"][q][:, k0 * H1:(k0 + k) * H1],
                    in_=st["tp"][:, 0:k * H1], func=COPY)
                st["tp"] = None
                st["tpn"] = 0

            def epi1(g):
                qn = min(4, NT - g * 4)
                qw = qn * 128
                gr = slice(g * 512, g * 512 + qw)
                tmpq = epp.tile([H1, 512], BF, tag="e1tmp")
                nc.vector.tensor_tensor(out=tmpq[:, 0:qw],
                                        in0=agg_sb[:, gr],
                                        in1=dinv_rep[:, gr], op=MULT)
                hq = epp.tile([H1, 512], BF, tag="e1h")
                nc.scalar.activation(out=hq[:, 0:qw], in_=tmpq[:, 0:qw],
                                     func=SILU, bias=b1c[:])
                nc.vector.tensor_add(out=r1T[:, gr], in0=hq[:, 0:qw],
                                     in1=r1T[:, gr])
                h2q = epp.tile([H1, 512], BF, tag="e1h2")
                nc.vector.tensor_tensor(out=h2q[:, 0:qw], in0=r1T[:, gr],
                                        in1=dinv_rep[:, gr], op=MULT)
                nc.scalar.activation(out=agg_sb[:, gr], in_=h2q[:, 0:qw],
                                     func=COPY)
                st = epi1_state
                for m in range(qn):
                    t = g * 4 + m
                    q = int(CHUNK_OF_TILE[t])
                    if st["cb"][q] is None:
                        st["cb"][q] = cbp.tile([128, 29 * H1], BF,
                                               tag="cb2", name=f"cb2_{q}")
                    tloc = t - int(CH_START[q])
                    if st["tp"] is None:
                        st["tp"] = ps_t.tile([128, 256], BF, tag="tpb",
                                             name=f"tpb{t}")
                        st["tp0"] = tloc
                        st["tpq"] = q
                    k = st["tpn"]
                    nc.tensor.matmul(out=st["tp"][:, k * H1:(k + 1) * H1],
                                     lhsT=h2q[:, m * 128:(m + 1) * 128],
                                     rhs=identb[0:H1, 0:H1],
                                     is_transpose=True, start=True, stop=True,
                                     skip_group_check=True)
                    st["tpn"] = k + 1
                    if st["tpn"] == 4 or tloc == int(CH_TILES[q]) - 1:
                        flush_tp(q)
                    st["done"][q] += 1
                    if st["done"][q] == int(CH_TILES[q]):
                        nc.sync.dma_start(
                            out=h2s_chunk[q][:].rearrange(
                                "(a p) c -> p a c", p=128)[:, :, 0:H1],
                            in_=st["cb"][q][:, :int(CH_TILES[q]) * H1]
                                .rearrange("p (a c) -> p a c", c=H1),
                            )
                        nc.gpsimd.collective_compute(
                            "AllGather", mybir.AluOpType.bypass,
                            replica_groups=[list(range(NCORES))],
                            ins=[h2s_chunk[q][:]],
                            outs=[table2[q][:]])

            run_layer(table1, epi1)

            # ---------------- layer 2 + pooling ----------------
            for g in range((NT + 3) // 4):
                qn = min(4, NT - g * 4)
                qw = qn * 128
                gr = slice(g * 512, g * 512 + qw)
                ps_r2 = ps_agg.tile([H1, 512], F32, tag="pagg",
                                    name=f"psr2{g}")
                for m in range(qn):
                    t = g * 4 + m
                    nc.tensor.matmul(
                        out=ps_r2[0:H2, m * 128:(m + 1) * 128],
                        lhsT=Wr2_sb[:],
                        rhs=r1T[:, t * 128:(t + 1) * 128],
                        start=True, stop=(m == qn - 1),
                        skip_group_check=True)
                nc.scalar.activation(out=r2T[:, gr], in_=ps_r2[0:H2, 0:qw],
                                     func=SILU, bias=br2c[:])
                nc.scalar.activation(out=r2T[:, gr], in_=r2T[:, gr],
                                     func=COPY, scale=float(alpha2))

            pool_ps = ps_pool.tile([H2, 256], F32, tag="pool")
            nc.vector.memset(pool_ps[:], 0.0)
            tcount = [0]
            epi2_state = {"tp": None, "zn": None, "tpn": 0, "t0": 0}

            def epi2(g):
                qn = min(4, NT - g * 4)
                qw = qn * 128
                gr = slice(g * 512, g * 512 + qw)
                a2q = epp.tile([H1, 512], BF, tag="e2a")
                nc.vector.tensor_tensor(out=a2q[:, 0:qw], in0=agg_sb[:, gr],
                                        in1=dinv_rep[:, gr], op=MULT)
                ps_zq = ps_agg.tile([H1, 512], F32, tag="pagg",
                                    name=f"psz{g}")
                for m in range(qn):
                    nc.tensor.matmul(
                        out=ps_zq[0:H2, m * 128:(m + 1) * 128],
                        lhsT=W2_sb[:],
                        rhs=a2q[:, m * 128:(m + 1) * 128],
                        start=True, stop=(m == qn - 1),
                        skip_group_check=True)
                zTq = epp.tile([H2, 512], BF, tag="e2z")
                nc.vector.tensor_add(out=zTq[:, 0:qw],
                                     in0=ps_zq[0:H2, 0:qw],
                                     in1=r2T[:, gr])
                ind4 = epp.tile([128, 4 * 256], BF, tag="e2ind")
                nc.vector.tensor_tensor(
                    out=ind4[:, :qn * 256].rearrange(
                        "p (m i) -> p m i", i=256),
                    in0=batchc[:, g * 4:g * 4 + qn].unsqueeze(2)
                        .to_broadcast([128, qn, 256]),
                    in1=iota256[:].unsqueeze(1).to_broadcast([128, qn, 256]),
                    op=ISEQ)
                tpz = ps_t.tile([128, 256], BF, tag="tpb", name=f"tpz{g}")
                for m in range(qn):
                    nc.tensor.matmul(out=tpz[:, m * H2:(m + 1) * H2],
                                     lhsT=zTq[:, m * 128:(m + 1) * 128],
                                     rhs=identb[0:H2, 0:H2],
                                     is_transpose=True, start=True, stop=True,
                                     skip_group_check=True)
                zn4 = epp.tile([128, 4 * H2], BF, tag="e2zn")
                nc.scalar.activation(out=zn4[:, 0:qn * H2],
                                     in_=tpz[:, 0:qn * H2], func=COPY)
                for m in range(qn):
                    t = g * 4 + m
                    nc.tensor.matmul(out=pool_ps[:],
                                     lhsT=zn4[:, m * H2:(m + 1) * H2],
                                     rhs=ind4[:, m * 256:(m + 1) * 256],
                                     start=False, stop=(t == NT - 1),
                                     skip_group_check=True)

            run_layer(table2, epi2)

            psums = hdp.tile([H2, 256], F32, tag="psums")
            nc.vector.tensor_copy(out=psums[:], in_=pool_ps[:])
            nc.sync.dma_start(out=pool_in[:], in_=psums[:])
            nc.gpsimd.collective_compute(
                "AllReduce", mybir.AluOpType.add,
                replica_groups=[list(range(NCORES))],
                ins=[pool_in[:]], outs=[pool_out[:]])

            # ---------------- head ----------------
            sums = hdp.tile([H2, 256], F32, tag="hsums")
            nc.sync.dma_start(out=sums[:], in_=pool_out[:])
            ge = hdp.tile([H2, 256], F32, tag="hge")
            nc.vector.tensor_tensor(out=ge[:], in0=sums[:], in1=invc[:],
                                    op=MULT)
            nc.vector.tensor_scalar_add(out=ge[:], in0=ge[:], scalar1=b2c[:])
            u_ps = ps_mm.tile([128, 256], F32, tag="mm", name="ups")
            nc.tensor.matmul(out=u_ps[0:80, 0:256], lhsT=Wf1a[:], rhs=ge[:],
                             start=True, stop=True)
            u = hdp.tile([80, 256], F32, tag="hu")
            nc.scalar.activation(out=u[:], in_=u_ps[0:80, 0:256], func=SILU,
                                 bias=mc_col[:])
            o_ps = ps_mm.tile([128, 256], F32, tag="mm", name="ops")
            nc.tensor.matmul(out=o_ps[0:NCLS, 0:256], lhsT=Wf2_sb[:],
                             rhs=u[:], start=True, stop=True)
            outT = hdp.tile([NCLS, 256], BF, tag="houtT")
            nc.vector.tensor_scalar_add(out=outT[:],
                                        in0=o_ps[0:NCLS, 0:256],
                                        scalar1=bf2c[:])
            for wdw in range(2):
                tp = ps_t.tile([128, 256], BF, tag="tpb", name=f"tpo{wdw}")
                nc.tensor.transpose(out=tp[:, 0:NCLS],
                                    in_=outT[:, wdw * 128:(wdw + 1) * 128],
                                    identity=identb[0:NCLS, 0:NCLS])
                ob = hdp.tile([128, NCLS], F32, tag="hob")
                nc.vector.tensor_copy(out=ob[:], in_=tp[:, 0:NCLS])
                nc.sync.dma_start(out=out[wdw * 128:(wdw + 1) * 128, :],
                                  in_=ob[:])

    nc.compile()
    return nc


def _host_metrics_contrib(tolerance, cost, time, quantity,
                          mW1, mb1, mW2, mb2, Wf1, bf1):
    silu = lambda v: v / (1.0 + np.exp(-v))
    m = np.stack([np.asarray(v, np.float32).reshape(1, 1) for v in
                  (tolerance, cost, time, quantity)])
    e = silu(np.einsum('gij,gjk->gik', m, np.asarray(mW1, np.float32))
             + np.asarray(mb1, np.float32)[:, None, :])
    e = (np.einsum('gij,gjk->gik', e, np.asarray(mW2, np.float32))
         + np.asarray(mb2, np.float32)[:, None, :])
    metvec = e.transpose(1, 0, 2).reshape(1, 64)
    mc = metvec @ np.asarray(Wf1, np.float32)[16:, :] \
        + np.asarray(bf1, np.float32)[None, :]
    return mc.astype(np.float32)


def kernel(x, edge_index, batch, tolerance, cost, time, quantity,
           W1, b1, W2, b2, Wr1, br1, Wr2, br2, alpha1, alpha2,
           mW1, mb1, mW2, mb2, Wf1, bf1, Wf2, bf2):
    x = np.asarray(x, np.float32)
    src = np.asarray(edge_index[0], np.int64)
    dst = np.asarray(edge_index[1], np.int64)
    batch = np.asarray(batch, np.int64)

    deg = 1.0 + np.bincount(dst, minlength=N).astype(np.float32)
    dinv_full = 1.0 / np.sqrt(deg)

    sched, per_core = _build_structure(src, dst)
    nc = _build_program(sched, float(alpha1), float(alpha2))

    iota128 = np.tile(np.arange(128, dtype=np.float32), (128, 1))
    iota256 = np.tile(np.arange(256, dtype=np.float32), (128, 1))

    cnts = np.bincount(batch, minlength=G).astype(np.float32)
    invc = 1.0 / np.maximum(cnts, 1.0)

    common = {
        "iota128": iota128.astype(BF16),
        "iota256": iota256.astype(BF16),
        "identb": np.eye(128, dtype=np.float32).astype(BF16),
        "identf": np.eye(128, dtype=np.float32),
        "W1": np.asarray(W1, np.float32).astype(BF16),
        "Wr1": np.asarray(Wr1, np.float32).astype(BF16),
        "W2": np.asarray(W2, np.float32).astype(BF16),
        "Wr2": np.asarray(Wr2, np.float32).astype(BF16),
        "b1c": np.asarray(b1, np.float32).reshape(H1, 1),
        "br1c": np.asarray(br1, np.float32).reshape(H1, 1),
        "b2c": np.asarray(b2, np.float32).reshape(H2, 1),
        "br2c": np.asarray(br2, np.float32).reshape(H2, 1),
        "Wf1a": np.asarray(Wf1[:16, :], np.float32),
        "Wf2": np.asarray(Wf2, np.float32),
        "mc_col": _host_metrics_contrib(
            tolerance, cost, time, quantity,
            mW1, mb1, mW2, mb2, Wf1, bf1).reshape(80, 1),
        "bf2c": np.asarray(bf2, np.float32).reshape(NCLS, 1),
        "invc_rep": np.tile(invc[None, :], (16, 1)),
    }

    in_maps = []
    for c in range(NCORES):
        lo, hi = c * SHARD, (c + 1) * SHARD
        xs = np.zeros((SHARD_PAD, CIN), np.float32)
        xs[:SHARD] = x[lo:hi]
        dv = np.zeros(SHARD_PAD, np.float32)
        dv[:SHARD] = dinv_full[lo:hi]
        bc = np.full(SHARD_PAD, -1.0, np.float32)
        bc[:SHARD] = batch[lo:hi].astype(np.float32)
        m = dict(common)
        m["xTb"] = np.ascontiguousarray(xs.T).astype(BF16)
        m["dinv_rep"] = np.tile(dv[None, :], (H1, 1)).astype(BF16)
        m["batchc"] = np.ascontiguousarray(
            bc.reshape(NT, 128).T)
        m["idx"] = per_core[c]["idx"]
        m["dstp"] = per_core[c]["dstp"]
        in_maps.append(m)

    res = run_bass_kernel_spmd(nc, in_maps, list(range(NCORES)))
    kernel._last = (nc, in_maps)
    kernel._res = res
    return np.asarray(res.results[0]["out"], np.float32)
